# revision 7
# baseline (speedup 1.0000x reference)
"""DarkChannelLoss Trainium2 kernel.

Computes mean((dark(real) - dark(fake))^2) where dark(x) is:
  x in [-1,1] -> (x+1)/2 -> channel min -> reflect-pad(7) -> 15x15 window min
  -> clip [0, 0.1]

Key identities used:
  * (x+1)/2 is monotone, so it commutes with all the mins: do every min in
    the raw domain and apply the affine at the very end (folded into the
    final scalar factor 0.25 on the host).
  * clip(v, 0, 0.1): the lower clip never binds ((m+1)/2 >= 0); the upper
    clip is min(m, -0.8) in the raw domain and commutes with the window
    mins, so it is fused into the last W-tree op.
  * reflect-pad(7) + VALID 15-window min == sliding min over the window
    [j-7, j+7] clamped to the image: every reflected index duplicates an
    in-window value.  Implemented by padding with +BIG and running a full
    15-window min.
  * 15-window sliding min via log tree of shifted pairwise mins
    (shifts 1, 2, 4, 7), separably over W then (after a PE transpose) H.

Sharding: pure data parallel, 2 images per core x 8 cores. Each core
returns per-partition partial sums of the squared raw clamped diff; the
host reduces and scales by 0.25 / (B*H*W).
"""

import sys

import numpy as np

for _p in ("/opt/trn_rl_repo",):
    if _p not in sys.path:
        sys.path.insert(0, _p)

import contextlib

import bass_rust
import concourse.bacc as bacc
import concourse.bass as bass
import concourse.mybir as mybir
from concourse import masks
from concourse.alu_op_type import AluOpType
from concourse.bass_utils import run_bass_kernel_spmd
from concourse.tile import TileContext

P = 128
H = 512
W = 512
C = 3
B = 16            # full batch
N_CORES = 8
B_LOCAL = B // N_CORES   # 2 images per core
TB = 2 * B_LOCAL         # real+fake images batched along the free dim
KP = 7                   # window radius (15 = 2*7+1)
WP = W + 2 * KP          # padded width 526
BIG = 1.0e30
CLAMP = -0.8             # raw-domain image of the 0.1 upper clip
F32 = mybir.dt.float32
MIN = AluOpType.min

_NC_CACHE = {}


def _build_nc():
    nc = bacc.Bacc(None)
    real = nc.declare_dram_parameter("real", [B_LOCAL, C, H, W], F32, isOutput=False)
    fake = nc.declare_dram_parameter("fake", [B_LOCAL, C, H, W], F32, isOutput=False)
    out = nc.declare_dram_parameter("out", [P, 1], F32, isOutput=True)

    n_hc = H // P   # 4 h-chunks
    n_wc = W // P   # 4 w-chunks

    with TileContext(nc) as tc, contextlib.ExitStack() as ctx:
        consts = ctx.enter_context(tc.tile_pool(name="consts", bufs=1))
        xin = ctx.enter_context(tc.tile_pool(name="xin", bufs=2))
        mp_pool = ctx.enter_context(tc.tile_pool(name="mp", bufs=2))
        tr_pool = ctx.enter_context(tc.tile_pool(name="tr", bufs=4))
        wout_pool = ctx.enter_context(tc.tile_pool(name="wout", bufs=n_hc))
        ps_pool = ctx.enter_context(
            tc.tile_pool(name="ps", bufs=2, space="PSUM")
        )
        th_pool = ctx.enter_context(tc.tile_pool(name="th", bufs=2))
        d_pool = ctx.enter_context(tc.tile_pool(name="d", bufs=2))
        pair_pool = ctx.enter_context(tc.tile_pool(name="pair", bufs=2))

        ident = consts.tile([P, P], F32)
        masks.make_identity(nc, ident[:])
        partials = consts.tile([P, n_wc], F32)

        def view(t, x):
            return t[:].rearrange("p (a x) -> p a x", a=TB, x=x)

        # ---------------- W phase: per h-chunk ----------------
        wouts = []
        for hc in range(n_hc):
            hs = hc * P
            X = xin.tile([P, TB * C * W], F32)  # [128, 6144]
            Xv = X[:].rearrange("p (a c w) -> p a c w", a=TB, c=C, w=W)
            nc.sync.dma_start(
                out=Xv[:, 0:B_LOCAL, :, :],
                in_=real[:, :, hs : hs + P, :].rearrange("b c h w -> h b c w"),
            )
            nc.sync.dma_start(
                out=Xv[:, B_LOCAL:TB, :, :],
                in_=fake[:, :, hs : hs + P, :].rearrange("b c h w -> h b c w"),
            )

            Mp = mp_pool.tile([P, TB * WP], F32)
            Mv = view(Mp, WP)
            nc.gpsimd.memset(Mv[:, :, 0:KP], BIG)
            nc.gpsimd.memset(Mv[:, :, W + KP : WP], BIG)
            nc.vector.tensor_tensor(
                Mv[:, :, KP : W + KP], Xv[:, :, 0, :], Xv[:, :, 1, :], MIN
            )
            nc.vector.tensor_tensor(
                Mv[:, :, KP : W + KP], Mv[:, :, KP : W + KP], Xv[:, :, 2, :], MIN
            )

            t2 = tr_pool.tile([P, TB * (WP - 1)], F32, tag="tr")
            t2v = view(t2, WP - 1)
            nc.vector.tensor_tensor(t2v, Mv[:, :, 0 : WP - 1], Mv[:, :, 1:WP], MIN)
            t4 = tr_pool.tile([P, TB * (WP - 3)], F32, tag="tr")
            t4v = view(t4, WP - 3)
            nc.vector.tensor_tensor(
                t4v, t2v[:, :, 0 : WP - 3], t2v[:, :, 2 : WP - 1], MIN
            )
            t8 = tr_pool.tile([P, TB * (WP - 7)], F32, tag="tr")
            t8v = view(t8, WP - 7)
            nc.vector.tensor_tensor(
                t8v, t4v[:, :, 0 : WP - 7], t4v[:, :, 4 : WP - 3], MIN
            )
            Wt = wout_pool.tile([P, TB * W], F32)
            Wv = view(Wt, W)
            # out = min(min(CLAMP, t8[j]), t8[j+7]) -- clamp fused for free
            nc.vector.scalar_tensor_tensor(
                Wv, t8v[:, :, 0:W], CLAMP, t8v[:, :, 7 : W + 7], MIN, MIN
            )
            wouts.append(Wt)

        # ---------------- H phase: per w-chunk ----------------
        for wc in range(n_wc):
            PT = ps_pool.tile([P, TB * H], F32)  # 4 PSUM banks
            for tb in range(TB):
                for hc in range(n_hc):
                    nc.tensor.transpose(
                        PT[:, tb * H + hc * P : tb * H + (hc + 1) * P],
                        wouts[hc][:, tb * W + wc * P : tb * W + (wc + 1) * P],
                        ident[:],
                    )
            TH = th_pool.tile([P, TB * WP], F32)
            THv = view(TH, WP)
            nc.gpsimd.memset(THv[:, :, 0:KP], BIG)
            nc.gpsimd.memset(THv[:, :, H + KP : WP], BIG)
            nc.scalar.copy(THv[:, :, KP : H + KP], view(PT, H))

            h2 = tr_pool.tile([P, TB * (WP - 1)], F32, tag="tr")
            h2v = view(h2, WP - 1)
            nc.vector.tensor_tensor(h2v, THv[:, :, 0 : WP - 1], THv[:, :, 1:WP], MIN)
            h4 = tr_pool.tile([P, TB * (WP - 3)], F32, tag="tr")
            h4v = view(h4, WP - 3)
            nc.vector.tensor_tensor(
                h4v, h2v[:, :, 0 : WP - 3], h2v[:, :, 2 : WP - 1], MIN
            )
            h8 = tr_pool.tile([P, TB * (WP - 7)], F32, tag="tr")
            h8v = view(h8, WP - 7)
            nc.vector.tensor_tensor(
                h8v, h4v[:, :, 0 : WP - 7], h4v[:, :, 4 : WP - 3], MIN
            )
            Dt = d_pool.tile([P, TB * H], F32)
            Dv = view(Dt, H)
            nc.vector.tensor_tensor(Dv, h8v[:, :, 0:H], h8v[:, :, 7 : H + 7], MIN)

            # pair stage: d = dark_r - dark_f ; accumulate sum(d^2)
            halfd = B_LOCAL * H  # 1024
            dd = pair_pool.tile([P, halfd], F32)
            nc.vector.tensor_tensor(
                dd[:], Dt[:, 0:halfd], Dt[:, halfd : 2 * halfd], AluOpType.subtract
            )
            d2 = pair_pool.tile([P, halfd], F32)
            nc.scalar.activation(
                d2[:],
                dd[:],
                bass_rust.ActivationFunctionType.Square,
                accum_out=partials[:, wc : wc + 1],
            )

        osb = consts.tile([P, 1], F32)
        nc.vector.tensor_reduce(
            osb[:], partials[:, 0:n_wc], axis=mybir.AxisListType.X, op=AluOpType.add
        )
        nc.sync.dma_start(out=out[:, :], in_=osb[:])

    return nc


def get_nc():
    if "nc" not in _NC_CACHE:
        nc = _build_nc()
        # bass2jax's pjrt path serializes nc.m directly and never calls
        # finalize(); Bacc defers register allocation to finalize().
        if not nc.is_finalized():
            nc.finalize()
        _NC_CACHE["nc"] = nc
    return _NC_CACHE["nc"]


def run_on_hw(real, fake, trace=False):
    """real/fake: [16,3,512,512] f32. Returns (per_core_results, BassKernelResults)."""
    nc = get_nc()
    real = np.ascontiguousarray(real, dtype=np.float32)
    fake = np.ascontiguousarray(fake, dtype=np.float32)
    in_maps = []
    for i in range(N_CORES):
        sl = slice(i * B_LOCAL, (i + 1) * B_LOCAL)
        in_maps.append({"real": real[sl], "fake": fake[sl]})
    res = run_bass_kernel_spmd(nc, in_maps, list(range(N_CORES)), trace=trace)
    return res


def kernel(real, fake):
    res = run_on_hw(real, fake, trace=False)
    total = 0.0
    for r in res.results:
        total += r["out"].astype(np.float64).sum()
    val = total * 0.25 / (B * H * W)
    return np.float32(val)
